# revision 2
# baseline (speedup 1.0000x reference)
"""CIN (Compressed Interaction Network) kernel for Trainium2, SPMD over 8 cores.

Reference computation (per layer l, with x0 = embeddings (B, M, D)):
    xk = relu(einsum("bmd,bhd,mhk->bkd", x0, x_{k-1}, W_l))   # (B, K, D)
    out_l = sum_d xk                                           # (B, K)
Output: concat(out_0, out_1, out_2) -> (B, 192)

Mapping (per core, B_loc = 2048 batch rows, data-parallel over B):
  * Interaction index (m,h) / output index k live on SBUF partitions;
    (b, d) is flattened on the free dim (N = BT*D per b-tile).
  * X0x4 (128, N): x0 rows tiled 4x vertically (host-prepped DRAM layout).
  * For each 128-row block g of the (m,h) interaction space:
      bc_g   = Sel_g.T @ X0x4     (PE -> PSUM; replicates x0[m(p)] into
                                   every partition p of the block)
      v_g    = bc_g * XkRep       (DVE tensor_tensor)
      out   += Wdup_g.T @ v_g     (PE, PSUM accumulation)
  * Wdup_g = [W_g | W_g] (host-duplicated) so the PSUM result lands
    duplicated in both partition halves -> after ReLU the SBUF tile is
    directly the next layer's replicated Xk (XkRep[p] = xk[p % 64]).
  * ReLU via ScalarE PSUM->SBUF; per-layer d-sums via DVE reduce; outputs
    stored k-major (192, B_loc) and transposed on host.

Self-contained: hardcodes shapes from the problem spec.
"""

import os

import ml_dtypes
import numpy as np

B, M, D = 16384, 32, 16
N_CORES = 8
B_LOC = B // N_CORES

BT = 64  # batch rows per b-tile
N_FREE = BT * D  # 1024 free elems per b-tile
N_TILES = B_LOC // BT
MM_FREE = 512  # one fp32 PSUM bank; max free dim per matmul

MODE = os.environ.get("CIN_MODE", "bf16")  # "bf16" | "f32r"

_CACHE = {}


def _np_dtype():
    return ml_dtypes.bfloat16 if MODE == "bf16" else np.float32


def _sel_matrix(g, m_per_block, h_per_block):
    """S (128, 128): S[p, q] = 1 iff X0x4[p] holds x0[m_g(q)].

    X0x4[p] = x0[p % 32]; column q of block g is m_g(q) = m_per_block*g +
    q // h_per_block. Rows use strip j = q // 32 to spread SBUF reads.
    """
    S = np.zeros((128, 128), dtype=np.float32)
    for q in range(128):
        m = m_per_block * g + q // h_per_block
        j = q // 32
        S[32 * j + (m % 32), q] = 1.0
    return S


def _prep_weights(W0, W1, W2):
    dt = _np_dtype()
    out = {}
    for i, W in enumerate((W0, W1, W2)):
        m, h, k = W.shape
        flat = np.ascontiguousarray(np.asarray(W, dtype=np.float32).reshape(m * h, k))
        G = (m * h) // 128
        blocks = flat.reshape(G, 128, k)
        dup = np.concatenate([blocks, blocks], axis=2)  # (G, 128, 128)
        out[f"w{i}dup"] = np.ascontiguousarray(dup.astype(dt))
    sel32 = np.stack([_sel_matrix(g, 4, 32) for g in range(8)])
    sel64 = np.stack([_sel_matrix(g, 2, 64) for g in range(16)])
    out["sel32"] = np.ascontiguousarray(sel32.astype(dt))
    out["sel64"] = np.ascontiguousarray(sel64.astype(dt))
    return out


def _build_bass():
    import concourse.bass as bass  # noqa: F401
    import concourse.mybir as mybir
    import concourse.tile as tile
    from concourse import bacc

    f32 = mybir.dt.float32
    if MODE == "bf16":
        dt_in = mybir.dt.bfloat16
        dt_mm = mybir.dt.bfloat16  # matmul view of dt_in-typed tiles
    else:
        dt_in = mybir.dt.float32r
        dt_mm = mybir.dt.float32r

    nc = bacc.Bacc(None, target_bir_lowering=False, debug=False)

    x0t4 = nc.dram_tensor("x0t4", (128, B_LOC * D), dt_in, kind="ExternalInput")
    w_dram = [
        nc.dram_tensor("w0dup", (8, 128, 128), dt_in, kind="ExternalInput"),
        nc.dram_tensor("w1dup", (16, 128, 128), dt_in, kind="ExternalInput"),
        nc.dram_tensor("w2dup", (16, 128, 128), dt_in, kind="ExternalInput"),
    ]
    s32_dram = nc.dram_tensor("sel32", (8, 128, 128), dt_in, kind="ExternalInput")
    s64_dram = nc.dram_tensor("sel64", (16, 128, 128), dt_in, kind="ExternalInput")
    out_dram = nc.dram_tensor("out", (192, B_LOC), f32, kind="ExternalOutput")

    with tile.TileContext(nc) as tc:
        with (
            tc.tile_pool(name="consts", bufs=1) as consts,
            tc.tile_pool(name="xin", bufs=3) as xin,
            tc.tile_pool(name="xk", bufs=3) as xkp,
            tc.tile_pool(name="vbuf", bufs=4) as vbuf,
            tc.tile_pool(name="obuf", bufs=4) as obuf,
            tc.tile_pool(name="bcps", bufs=3, space="PSUM") as bcps,
            tc.tile_pool(name="outps", bufs=1, space="PSUM") as outps,
        ):
            w_sb = []
            for i, (wd, G) in enumerate(zip(w_dram, (8, 16, 16))):
                t = consts.tile([128, G, 128], dt_in, tag=f"w{i}")
                nc.sync.dma_start(out=t, in_=wd.rearrange("g p q -> p g q"))
                w_sb.append(t)
            s32_sb = consts.tile([128, 8, 128], dt_in, tag="s32")
            nc.sync.dma_start(out=s32_sb, in_=s32_dram.rearrange("g p q -> p g q"))
            s64_sb = consts.tile([128, 16, 128], dt_in, tag="s64")
            nc.sync.dma_start(out=s64_sb, in_=s64_dram.rearrange("g p q -> p g q"))
            sel_sb = [s32_sb, s64_sb, s64_sb]

            for t_i in range(N_TILES):
                xt = xin.tile([128, N_FREE], dt_in, tag="x0x4")
                nc.sync.dma_start(
                    out=xt, in_=x0t4[:, t_i * N_FREE : (t_i + 1) * N_FREE]
                )

                xk_rep = xt  # layer 0: XkRep[p] = x0[p % 32] == X0x4
                for layer in range(3):
                    G = 8 if layer == 0 else 16
                    W = w_sb[layer]
                    S = sel_sb[layer]

                    ops = outps.tile([128, N_FREE], f32, tag="outps")
                    for g in range(G):
                        bc = bcps.tile([128, N_FREE], f32, tag="bc")
                        for h0 in range(0, N_FREE, MM_FREE):
                            nc.tensor.matmul(
                                bc[:, h0 : h0 + MM_FREE],
                                lhsT=S[:, g, :],
                                rhs=xt[:, h0 : h0 + MM_FREE],
                                start=True,
                                stop=True,
                            )
                        v = vbuf.tile([128, N_FREE], dt_in, tag="v")
                        nc.vector.tensor_mul(v, bc, xk_rep)
                        for h0 in range(0, N_FREE, MM_FREE):
                            nc.tensor.matmul(
                                ops[:, h0 : h0 + MM_FREE],
                                lhsT=W[:, g, :],
                                rhs=v[:, h0 : h0 + MM_FREE],
                                start=(g == 0),
                                stop=(g == G - 1),
                            )

                    xk_new = xkp.tile([128, N_FREE], dt_in, tag="xk")
                    nc.scalar.activation(
                        xk_new, ops, mybir.ActivationFunctionType.Relu
                    )
                    outs = obuf.tile([64, BT], f32, tag="outs")
                    nc.vector.reduce_sum(
                        out=outs,
                        in_=xk_new[:64].rearrange("k (b d) -> k b d", d=D),
                        axis=mybir.AxisListType.X,
                    )
                    nc.sync.dma_start(
                        out=out_dram[
                            layer * 64 : (layer + 1) * 64,
                            t_i * BT : (t_i + 1) * BT,
                        ],
                        in_=outs,
                    )
                    xk_rep = xk_new

    nc.finalize()
    return nc


def _get_program():
    if "nc" not in _CACHE:
        _CACHE["nc"] = _build_bass()
    return _CACHE["nc"]


def kernel(embeddings, W0, W1, W2):
    from concourse.bass_utils import run_bass_kernel_spmd

    dt = _np_dtype()
    embeddings = np.asarray(embeddings, dtype=np.float32)
    wmaps = _prep_weights(np.asarray(W0), np.asarray(W1), np.asarray(W2))

    in_maps = []
    for c in range(N_CORES):
        emb = embeddings[c * B_LOC : (c + 1) * B_LOC]  # (B_LOC, M, D)
        x0t = np.ascontiguousarray(emb.transpose(1, 0, 2).reshape(M, B_LOC * D))
        x0t4 = np.ascontiguousarray(np.tile(x0t, (4, 1)).astype(dt))
        in_maps.append({"x0t4": x0t4, **wmaps})

    nc = _get_program()
    res = run_bass_kernel_spmd(
        nc,
        in_maps,
        core_ids=list(range(N_CORES)),
        trace=bool(int(os.environ.get("CIN_TRACE", "0"))),
    )
    if res.exec_time_ns is not None:
        _CACHE["exec_time_ns"] = res.exec_time_ns

    outs = [r["out"].T for r in res.results]  # each (B_LOC, 192)
    return np.ascontiguousarray(np.concatenate(outs, axis=0))


# revision 3
# speedup vs baseline: 83.2310x; 83.2310x over previous
"""CIN (Compressed Interaction Network) kernel for Trainium2, SPMD over 8 cores.

Reference computation (per layer l, with x0 = embeddings (B, M, D)):
    xk = relu(einsum("bmd,bhd,mhk->bkd", x0, x_{k-1}, W_l))   # (B, K, D)
    out_l = sum_d xk                                           # (B, K)
Output: concat(out_0, out_1, out_2) -> (B, 192)

Mapping (per core, B_loc = 2048 batch rows, data-parallel over B):
  * Interaction index (m,h) / output index k live on SBUF partitions;
    (b, d) is flattened on the free dim (N = BT*D per b-tile).
  * X0x4 (128, N): x0 rows tiled 4x vertically (host-prepped DRAM layout).
  * For each 128-row block g of the (m,h) interaction space:
      bc_g   = Sel_g.T @ X0x4     (PE -> PSUM; replicates x0[m(p)] into
                                   every partition p of the block)
      v_g    = bc_g * XkRep       (DVE tensor_tensor)
      out   += Wdup_g.T @ v_g     (PE, PSUM accumulation)
  * Wdup_g = [W_g | W_g] (host-duplicated) so the PSUM result lands
    duplicated in both partition halves -> after ReLU the SBUF tile is
    directly the next layer's replicated Xk (XkRep[p] = xk[p % 64]).
  * ReLU via ScalarE PSUM->SBUF; per-layer d-sums via DVE reduce; outputs
    stored k-major (192, B_loc) and transposed on host.

Self-contained: hardcodes shapes from the problem spec.
"""

import os

import ml_dtypes
import numpy as np

B, M, D = 16384, 32, 16
N_CORES = 8
B_LOC = B // N_CORES

BT = 64  # batch rows per b-tile
N_FREE = BT * D  # 1024 free elems per b-tile
N_TILES = B_LOC // BT
MM_FREE = 512  # one fp32 PSUM bank; max free dim per matmul

MODE = os.environ.get("CIN_MODE", "bf16")  # "bf16" | "f32r"

_CACHE = {}


def _np_dtype():
    return ml_dtypes.bfloat16 if MODE == "bf16" else np.float32


def _sel_matrix(g, m_per_block, h_per_block):
    """S (128, 128): S[p, q] = 1 iff X0x4[p] holds x0[m_g(q)].

    X0x4[p] = x0[p % 32]; column q of block g is m_g(q) = m_per_block*g +
    q // h_per_block. Rows use strip j = q // 32 to spread SBUF reads.
    """
    S = np.zeros((128, 128), dtype=np.float32)
    for q in range(128):
        m = m_per_block * g + q // h_per_block
        j = q // 32
        S[32 * j + (m % 32), q] = 1.0
    return S


def _prep_weights(W0, W1, W2):
    dt = _np_dtype()
    out = {}
    for i, W in enumerate((W0, W1, W2)):
        m, h, k = W.shape
        flat = np.ascontiguousarray(np.asarray(W, dtype=np.float32).reshape(m * h, k))
        G = (m * h) // 128
        blocks = flat.reshape(G, 128, k)
        dup = np.concatenate([blocks, blocks], axis=2)  # (G, 128, 128)
        out[f"w{i}dup"] = np.ascontiguousarray(dup.astype(dt))
    sel32 = np.stack([_sel_matrix(g, 4, 32) for g in range(8)])
    sel64 = np.stack([_sel_matrix(g, 2, 64) for g in range(16)])
    out["sel32"] = np.ascontiguousarray(sel32.astype(dt))
    out["sel64"] = np.ascontiguousarray(sel64.astype(dt))
    return out


def _build_bass():
    import concourse.bass as bass  # noqa: F401
    import concourse.mybir as mybir
    import concourse.tile as tile
    from concourse import bacc

    f32 = mybir.dt.float32
    if MODE == "bf16":
        dt_in = mybir.dt.bfloat16
        dt_mm = mybir.dt.bfloat16  # matmul view of dt_in-typed tiles
    else:
        dt_in = mybir.dt.float32r
        dt_mm = mybir.dt.float32r

    nc = bacc.Bacc(None, target_bir_lowering=False, debug=False)

    x0t4 = nc.dram_tensor("x0t4", (128, B_LOC * D), dt_in, kind="ExternalInput")
    w_dram = [
        nc.dram_tensor("w0dup", (8, 128, 128), dt_in, kind="ExternalInput"),
        nc.dram_tensor("w1dup", (16, 128, 128), dt_in, kind="ExternalInput"),
        nc.dram_tensor("w2dup", (16, 128, 128), dt_in, kind="ExternalInput"),
    ]
    s32_dram = nc.dram_tensor("sel32", (8, 128, 128), dt_in, kind="ExternalInput")
    s64_dram = nc.dram_tensor("sel64", (16, 128, 128), dt_in, kind="ExternalInput")
    out_dram = nc.dram_tensor("out", (192, B_LOC), f32, kind="ExternalOutput")

    with tile.TileContext(nc) as tc:
        with (
            tc.tile_pool(name="consts", bufs=1) as consts,
            tc.tile_pool(name="xin", bufs=3) as xin,
            tc.tile_pool(name="xk", bufs=3) as xkp,
            tc.tile_pool(name="vbuf", bufs=4) as vbuf,
            tc.tile_pool(name="obuf", bufs=4) as obuf,
            tc.tile_pool(name="bcps", bufs=3, space="PSUM") as bcps,
            tc.tile_pool(name="outps", bufs=1, space="PSUM") as outps,
        ):
            w_sb = []
            for i, (wd, G) in enumerate(zip(w_dram, (8, 16, 16))):
                t = consts.tile([128, G, 128], dt_in, tag=f"w{i}")
                nc.sync.dma_start(out=t, in_=wd.rearrange("g p q -> p g q"))
                w_sb.append(t)
            s32_sb = consts.tile([128, 8, 128], dt_in, tag="s32")
            nc.sync.dma_start(out=s32_sb, in_=s32_dram.rearrange("g p q -> p g q"))
            s64_sb = consts.tile([128, 16, 128], dt_in, tag="s64")
            nc.sync.dma_start(out=s64_sb, in_=s64_dram.rearrange("g p q -> p g q"))
            sel_sb = [s32_sb, s64_sb, s64_sb]

            for t_i in range(N_TILES):
                xt = xin.tile([128, N_FREE], dt_in, tag="x0x4")
                nc.sync.dma_start(
                    out=xt, in_=x0t4[:, t_i * N_FREE : (t_i + 1) * N_FREE]
                )

                xk_rep = xt  # layer 0: XkRep[p] = x0[p % 32] == X0x4
                for layer in range(3):
                    G = 8 if layer == 0 else 16
                    W = w_sb[layer]
                    S = sel_sb[layer]

                    ops = outps.tile([128, N_FREE], f32, tag="outps")
                    for g in range(G):
                        bc = bcps.tile([128, N_FREE], f32, tag="bc")
                        for h0 in range(0, N_FREE, MM_FREE):
                            nc.tensor.matmul(
                                bc[:, h0 : h0 + MM_FREE],
                                lhsT=S[:, g, :],
                                rhs=xt[:, h0 : h0 + MM_FREE],
                                start=True,
                                stop=True,
                            )
                        v = vbuf.tile([128, N_FREE], dt_in, tag="v")
                        nc.vector.tensor_mul(v, bc, xk_rep)
                        for h0 in range(0, N_FREE, MM_FREE):
                            nc.tensor.matmul(
                                ops[:, h0 : h0 + MM_FREE],
                                lhsT=W[:, g, :],
                                rhs=v[:, h0 : h0 + MM_FREE],
                                start=(g == 0),
                                stop=(g == G - 1),
                            )

                    xk_new = xkp.tile([128, N_FREE], dt_in, tag="xk")
                    nc.scalar.activation(
                        xk_new, ops, mybir.ActivationFunctionType.Relu
                    )
                    outs = obuf.tile([64, BT], f32, tag="outs")
                    nc.vector.reduce_sum(
                        out=outs,
                        in_=xk_new[:64].rearrange("k (b d) -> k b d", d=D),
                        axis=mybir.AxisListType.X,
                    )
                    nc.sync.dma_start(
                        out=out_dram[
                            layer * 64 : (layer + 1) * 64,
                            t_i * BT : (t_i + 1) * BT,
                        ],
                        in_=outs,
                    )
                    xk_rep = xk_new

    nc.finalize()
    return nc


def _get_program():
    if "nc" not in _CACHE:
        _CACHE["nc"] = _build_bass()
    return _CACHE["nc"]


def kernel(embeddings, W0, W1, W2):
    from concourse.bass_utils import run_bass_kernel_spmd

    dt = _np_dtype()
    embeddings = np.asarray(embeddings, dtype=np.float32)
    wmaps = _prep_weights(np.asarray(W0), np.asarray(W1), np.asarray(W2))

    in_maps = []
    for c in range(N_CORES):
        emb = embeddings[c * B_LOC : (c + 1) * B_LOC]  # (B_LOC, M, D)
        x0t = np.ascontiguousarray(emb.transpose(1, 0, 2).reshape(M, B_LOC * D))
        x0t4 = np.ascontiguousarray(np.tile(x0t, (4, 1)).astype(dt))
        in_maps.append({"x0t4": x0t4, **wmaps})

    nc = _get_program()
    res = run_bass_kernel_spmd(nc, in_maps, core_ids=list(range(N_CORES)))
    if res.exec_time_ns is not None:
        _CACHE["exec_time_ns"] = res.exec_time_ns

    outs = [r["out"].T for r in res.results]  # each (B_LOC, 192)
    return np.ascontiguousarray(np.concatenate(outs, axis=0))
